# revision 1
# baseline (speedup 1.0000x reference)
"""AffinityLoss Bass kernel for 8 TRN2 NeuronCores (data-parallel over batch).

Math (validated vs reference in numpy, rel err ~3e-6):
  loss = sum_b |S_b|^2 / (sum_b c_b^2 + 1), S_b = sum of unit radial normals
  over selected contacts, c_b = #selected.  (cos_sim*mask).sum() == |sum m n|^2.
  Selection: 10 smallest of 126 per-(kp,face) grid-min distances (squared-form,
  monotone), masked by d < 0.2*length, exact tie handling by prefix count.
  Grid min per (kp,face): for each of 11 u-columns the quadratic in v is
  minimized exactly at v = clamp(round(10*vstar))/10, vstar = (a.h - a.b)/|a|^2.
  Column argmin extracted by tagged min over KI = k + 16*iu (exact in bf16).

Layouts per core (bs=1024):
  B-layout [128 batch partitions, feat] x 8 tiles : input, features, selection
  T-layout [pair/feature partitions, 1024 b]      : everything else
"""

import sys
import numpy as np

for _p in ("/opt/trn_rl_repo", "/root/.axon_site/_ro/trn_rl_repo"):
    if _p not in sys.path:
        sys.path.append(_p)

import concourse.bass as bass
import concourse.bacc as bacc
import concourse.mybir as mybir
import concourse.tile as tile
from concourse import bass_utils
from concourse.mybir import AluOpType as alu
from concourse.mybir import ActivationFunctionType as act

F32 = mybir.dt.float32
BF16 = mybir.dt.bfloat16
AX = mybir.AxisListType

N_KP, N_C, N_F, N_IU = 21, 8, 6, 11
N_PAIR = N_KP * N_F            # 126
B_CORE = 1024
N_CORES = 8
N_TILES = B_CORE // 128
MAGIC = 8388608.0              # 2^23: round-to-nearest via add/sub (f32)
TAGBIG = 256.0                 # argmin tag offset (exact in bf16)

FACE = np.array([[0, 1, 2, 3], [0, 4, 2, 6], [0, 1, 4, 5],
                 [1, 3, 5, 7], [2, 3, 6, 7], [4, 5, 6, 7]])
US = np.linspace(0.0, 1.0, N_IU)

# T-layout feature chunks:
# FT1 rows: G[kp,c] kp 0..15 (row kp*8+c)
# FT2 rows: 0..39 G kp16..20 | 40..103 M[p,q] | 104..124 HH[kp] | 125..127 zero
# FT3 rows: poses flat (hand kp*3+x ; corner 63+c*3+x) | 87.. zero


def _g_row(kp, c):
    r = kp * 8 + c
    return (0, r) if kp < 16 else (1, r - 128)


def _m_row(p, q):
    return 40 + p * 8 + q


def build_consts():
    w_s1 = np.zeros((N_IU, 2, 128, N_PAIR), np.float64)
    w_c = np.zeros((N_IU, 2, 128, N_PAIR), np.float64)
    w_aa = np.zeros((128, 66), np.float64)
    w_rep = np.zeros((N_IU, 66, N_PAIR), np.float64)
    for f in range(N_F):
        F0, F1, F2, F3 = FACE[f]
        for iu in range(N_IU):
            u = US[iu]; w0 = 1.0 - u
            col66 = f * N_IU + iu
            for (p, q, s) in [(F0, F0, w0 * w0), (F0, F2, -2 * w0 * w0), (F2, F2, w0 * w0),
                              (F1, F1, u * u), (F1, F3, -2 * u * u), (F3, F3, u * u),
                              (F0, F1, 2 * u * w0), (F0, F3, -2 * u * w0),
                              (F2, F1, -2 * u * w0), (F2, F3, 2 * u * w0)]:
                w_aa[_m_row(p, q), col66] += s
            ab_terms = [(F0, F2, w0 * w0), (F0, F3, w0 * u),
                        (F2, F2, -w0 * w0), (F2, F3, -w0 * u),
                        (F1, F2, u * w0), (F1, F3, u * u),
                        (F3, F2, -u * w0), (F3, F3, -u * u)]
            bb_terms = [(F2, F2, w0 * w0), (F2, F3, 2 * w0 * u), (F3, F3, u * u)]
            for kp in range(N_KP):
                col = kp * N_F + f
                w_rep[iu, col66, col] = 1.0
                for (c, s) in [(F0, w0), (F2, -w0), (F1, u), (F3, -u)]:
                    ch, r = _g_row(kp, c)
                    w_s1[iu, ch, r, col] += s
                for (p, q, s) in ab_terms:      # S1 -= a.b  (M rows live in FT2)
                    w_s1[iu, 1, _m_row(p, q), col] += -s
                for (c, s) in [(F2, -2 * w0), (F3, -2 * u)]:
                    ch, r = _g_row(kp, c)
                    w_c[iu, ch, r, col] += s
                for (p, q, s) in bb_terms:      # C += bb
                    w_c[iu, 1, _m_row(p, q), col] += s
                w_c[iu, 1, 104 + kp, col] += 1.0

    # pair geometry from FT3: kinds T1=c2, T2=c3-c2, T3=c0-c2, T4=c1-c3-c0+c2,
    # P1=mean(c0..3), DV=p2-p1 ; per component x
    w_geo = np.zeros((3, 6, 128, N_PAIR), np.float64)
    for f in range(N_F):
        F0, F1, F2, F3 = FACE[f]
        for x in range(3):
            row = {c: 63 + 3 * c + x for c in range(8)}
            for kp in range(N_KP):
                col = kp * N_F + f
                w_geo[x, 0, row[F2], col] += 1.0
                for c in range(4):
                    w_geo[x, 0, row[c], col] += -0.25   # T1 = c2 - p1
                w_geo[x, 1, row[F3], col] += 1.0
                w_geo[x, 1, row[F2], col] -= 1.0
                w_geo[x, 2, row[F0], col] += 1.0
                w_geo[x, 2, row[F2], col] -= 1.0
                w_geo[x, 3, row[F1], col] += 1.0
                w_geo[x, 3, row[F3], col] -= 1.0
                w_geo[x, 3, row[F0], col] -= 1.0
                w_geo[x, 3, row[F2], col] += 1.0
                for c in range(4):
                    w_geo[x, 4, row[c], col] += 0.25
                    w_geo[x, 5, row[c], col] -= 0.25
                for c in range(4, 8):
                    w_geo[x, 5, row[c], col] += 0.25

    w_stats = np.zeros((128, 32), np.float64)   # K=FT2; row0 dvn2, 1..8 edges^2
    for i in range(4):
        for j in range(4):
            w_stats[_m_row(i, j), 0] += 1.0 / 16
            w_stats[_m_row(i + 4, j + 4), 0] += 1.0 / 16
            w_stats[_m_row(i, j + 4), 0] += -1.0 / 16
            w_stats[_m_row(i + 4, j), 0] += -1.0 / 16
    edges = [(0, 1), (1, 2), (2, 3), (3, 0), (4, 5), (5, 6), (6, 7), (7, 4)]
    for e, (i, j) in enumerate(edges):
        w_stats[_m_row(i, i), 1 + e] += 1.0
        w_stats[_m_row(j, j), 1 + e] += 1.0
        w_stats[_m_row(i, j), 1 + e] += -1.0
        w_stats[_m_row(j, i), 1 + e] += -1.0

    w_tau = np.zeros((32, 1), np.float64)
    w_tau[1:9, 0] = 0.025

    ones126 = np.ones((126, 1), np.float32)
    ones_r = np.ones((1, 126), np.float32)
    return {
        "w_s1": w_s1.astype(np.float32).reshape(N_IU * 2 * 128, N_PAIR),
        "w_c": w_c.astype(np.float32).reshape(N_IU * 2 * 128, N_PAIR),
        "w_a3": w_aa.astype(np.float32),
        "w_rep": w_rep.astype(np.float32).reshape(N_IU * 66, N_PAIR),
        "w_geo": w_geo.astype(np.float32).reshape(18 * 128, N_PAIR),
        "w_stats": w_stats.astype(np.float32),
        "w_tau": w_tau.astype(np.float32),
        "ident": np.eye(128, dtype=np.float32),
        "ones126": ones126,
        "ones_r": ones_r,
    }


def build_kernel(nc: bass.Bass):
    ap = {}
    ap["poses"] = nc.dram_tensor("poses", [B_CORE, 87], F32, kind="ExternalInput").ap()
    for name, shape in [("w_s1", [N_IU * 2 * 128, N_PAIR]), ("w_c", [N_IU * 2 * 128, N_PAIR]),
                        ("w_a3", [128, 66]), ("w_rep", [N_IU * 66, N_PAIR]),
                        ("w_geo", [18 * 128, N_PAIR]), ("w_stats", [128, 32]),
                        ("w_tau", [32, 1]), ("ident", [128, 128]), ("ones126", [126, 1]),
                        ("ones_r", [1, 126])]:
        ap[name] = nc.dram_tensor(name, shape, F32, kind="ExternalInput").ap()
    ap["out"] = nc.dram_tensor("out", [2, B_CORE], F32, kind="ExternalOutput").ap()

    with tile.TileContext(nc) as tc:
        _emit(nc, tc, ap)
    return nc


def _emit(nc, tc, d):
    import contextlib
    ctx = contextlib.ExitStack()
    cpool = ctx.enter_context(tc.tile_pool(name="consts", bufs=1))
    wpool = ctx.enter_context(tc.tile_pool(name="wstream", bufs=2))
    bpool = ctx.enter_context(tc.tile_pool(name="blay", bufs=3))
    tpool = ctx.enter_context(tc.tile_pool(name="tlay", bufs=1))
    colpool = ctx.enter_context(tc.tile_pool(name="col", bufs=3))

    # ---------------- resident consts ----------------
    ident = cpool.tile([128, 128], F32, tag="ident")
    nc.sync.dma_start(out=ident[:, :], in_=d["ident"])
    w_a3 = cpool.tile([128, 66], F32, tag="w_a3")
    nc.sync.dma_start(out=w_a3[:, :], in_=d["w_a3"])
    w_rep = cpool.tile([66, N_IU * N_PAIR], F32, tag="w_rep")
    nc.sync.dma_start(out=w_rep[:, :].rearrange("k (i m) -> k i m", i=N_IU),
                      in_=d["w_rep"].rearrange("(i k) m -> k i m", i=N_IU))
    w_geo = cpool.tile([128, 18 * N_PAIR], F32, tag="w_geo")
    nc.sync.dma_start(out=w_geo[:, :].rearrange("k (g m) -> k g m", g=18),
                      in_=d["w_geo"].rearrange("(g k) m -> k g m", g=18))
    w_stats = cpool.tile([128, 32], F32, tag="w_stats")
    nc.sync.dma_start(out=w_stats[:, :], in_=d["w_stats"])
    w_tau = cpool.tile([32, 1], F32, tag="w_tau")
    nc.sync.dma_start(out=w_tau[:, :], in_=d["w_tau"])
    ones126 = cpool.tile([126, 1], F32, tag="ones126")
    nc.sync.dma_start(out=ones126[:, :], in_=d["ones126"])
    ones_r = cpool.tile([1, 126], F32, tag="ones_r")
    nc.sync.dma_start(out=ones_r[:, :], in_=d["ones_r"])
    c10 = cpool.tile([128, 1], F32, tag="c10")
    nc.vector.memset(c10[:, :], 10.0)
    nc.const_aps.aps[(F32, 10.0)] = c10[:, :]

    ft1 = tpool.tile([128, B_CORE], F32, tag="ft1")
    ft2 = tpool.tile([128, B_CORE], F32, tag="ft2")
    ft3 = tpool.tile([128, B_CORE], F32, tag="ft3")

    # ---------------- B-stage: features + transpose ----------------
    with tc.tile_pool(name="psA", bufs=2, space="PSUM") as psA:
        for t in range(N_TILES):
            pb = bpool.tile([128, 128], F32, tag="poseb")
            nc.vector.memset(pb[:, 87:128], 0.0)
            nc.sync.dma_start(out=pb[:, 0:87], in_=d["poses"][t * 128:(t + 1) * 128, :])
            fb = bpool.tile([128, 256], F32, tag="featb")
            nc.vector.memset(fb[:, 253:256], 0.0)
            h_ap = pb[:, 0:63].rearrange("p (k x) -> p k x", x=3)
            o_ap = pb[:, 63:87].rearrange("p (c x) -> p c x", x=3)
            # G[kp,c]
            sc1 = bpool.tile([128, 504], F32, tag="sc1")
            nc.vector.tensor_tensor(sc1[:, :].rearrange("p (k c x) -> p k c x", c=8, x=3),
                                    h_ap.unsqueeze(2).to_broadcast([128, 21, 8, 3]),
                                    o_ap.unsqueeze(1).to_broadcast([128, 21, 8, 3]),
                                    op=alu.mult)
            nc.vector.tensor_reduce(fb[:, 0:168].rearrange("p (k c) -> p k c", c=8),
                                    sc1[:, :].rearrange("p (k c x) -> p k c x", c=8, x=3),
                                    axis=AX.X, op=alu.add)
            # M[p,q]
            sc2 = bpool.tile([128, 192], F32, tag="sc2")
            nc.vector.tensor_tensor(sc2[:, :].rearrange("p (a b x) -> p a b x", b=8, x=3),
                                    o_ap.unsqueeze(2).to_broadcast([128, 8, 8, 3]),
                                    o_ap.unsqueeze(1).to_broadcast([128, 8, 8, 3]),
                                    op=alu.mult)
            nc.vector.tensor_reduce(fb[:, 168:232].rearrange("p (a b) -> p a b", b=8),
                                    sc2[:, :].rearrange("p (a b x) -> p a b x", b=8, x=3),
                                    axis=AX.X, op=alu.add)
            # HH[kp]
            sc3 = bpool.tile([128, 63], F32, tag="sc3")
            nc.vector.tensor_tensor(sc3[:, :].rearrange("p (k x) -> p k x", x=3),
                                    h_ap, h_ap, op=alu.mult)
            nc.vector.tensor_reduce(fb[:, 232:253].rearrange("p k -> p k"),
                                    sc3[:, :].rearrange("p (k x) -> p k x", x=3),
                                    axis=AX.X, op=alu.add)
            for (src, dst) in ((fb[:, 0:128], ft1), (fb[:, 128:256], ft2), (pb[:, :], ft3)):
                pt = psA.tile([128, 128], F32, tag="tpose")
                nc.tensor.transpose(out=pt[:, :], in_=src, identity=ident[:, :])
                nc.scalar.activation(dst[:, t * 128:(t + 1) * 128], pt[:, :], act.Copy)

    # ---------------- T-stage precomputes ----------------
    a3sb = tpool.tile([66, B_CORE], F32, tag="a3sb")   # A66
    ra66 = tpool.tile([66, B_CORE], F32, tag="ra66")
    stats_sq = tpool.tile([32, B_CORE], F32, tag="stats_sq")
    tau2_t = tpool.tile([1, B_CORE], F32, tag="tau2_t")
    rdvn2r = tpool.tile([126, B_CORE], F32, tag="rdvn2r")
    with tc.tile_pool(name="psB", bufs=2, space="PSUM") as psB:
        for h in range(2):
            bs = slice(h * 512, (h + 1) * 512)
            ps = psB.tile([66, 512], F32, tag="ps_a3")
            nc.tensor.matmul(ps[:, :], lhsT=w_a3[:, :], rhs=ft2[:, bs],
                             start=True, stop=True)
            nc.scalar.activation(a3sb[:, bs], ps[:, :], act.Copy)
            ps2 = psB.tile([32, 512], F32, tag="ps_st")
            nc.tensor.matmul(ps2[:, :], lhsT=w_stats[:, :], rhs=ft2[:, bs],
                             start=True, stop=True)
            nc.scalar.activation(stats_sq[:, bs], ps2[:, :], act.Sqrt)
        nc.vector.reciprocal_approx_fast(out=ra66[:, :], in_=a3sb[:, :])
        for h in range(2):
            bs = slice(h * 512, (h + 1) * 512)
            ps3 = psB.tile([1, 512], F32, tag="ps_tau")
            nc.tensor.matmul(ps3[:, :], lhsT=w_tau[:, :], rhs=stats_sq[:, bs],
                             start=True, stop=True)
            nc.scalar.activation(tau2_t[:, bs], ps3[:, :], act.Square)
        nc.vector.tensor_scalar_add(tau2_t[:, :], tau2_t[:, :], -1e-6)
        # rdvn2 = (1/(dvn+1e-5))^2, replicated to 126 rows
        rdvn2 = tpool.tile([1, B_CORE], F32, tag="rdvn2")
        nc.vector.tensor_scalar_add(rdvn2[:, :], stats_sq[0:1, :], 1e-5)
        nc.vector.reciprocal_approx_fast(out=rdvn2[:, :], in_=rdvn2[:, :])
        nc.vector.tensor_tensor(rdvn2[:, :], rdvn2[:, :], rdvn2[:, :], op=alu.mult)
        for h in range(2):
            bs = slice(h * 512, (h + 1) * 512)
            ps4 = psB.tile([126, 512], F32, tag="ps_rd")
            nc.tensor.matmul(ps4[:, :], lhsT=ones_r[:, :], rhs=rdvn2[:, bs],
                             start=True, stop=True)
            nc.scalar.activation(rdvn2r[:, bs], ps4[:, :], act.Copy)

    # ---------------- column stage ----------------
    mrun = tpool.tile([126, B_CORE], F32, tag="mrun")
    nc.vector.memset(mrun[:, :], 3.0e38)
    ctrun = tpool.tile([126, B_CORE], F32, tag="ctrun")
    nc.vector.memset(ctrun[:, :], 0.0)

    with tc.tile_pool(name="psC", bufs=2, space="PSUM") as psC:
        for iu in range(N_IU):
            ws1 = wpool.tile([128, 2 * N_PAIR], F32, tag="ws1")
            nc.sync.dma_start(out=ws1[:, :].rearrange("k (c m) -> k c m", c=2),
                              in_=d["w_s1"].rearrange("(i c k) m -> i k c m",
                                                      i=N_IU, c=2)[iu])
            wc = wpool.tile([128, 2 * N_PAIR], F32, tag="wc")
            nc.sync.dma_start(out=wc[:, :].rearrange("k (c m) -> k c m", c=2),
                              in_=d["w_c"].rearrange("(i c k) m -> i k c m",
                                                     i=N_IU, c=2)[iu])
            rep = w_rep[:, iu * N_PAIR:(iu + 1) * N_PAIR]
            for h in range(2):
                bs = slice(h * 512, (h + 1) * 512)
                s1p = psC.tile([126, 512], F32, tag="s1p")
                ccp = psC.tile([126, 512], F32, tag="ccp")
                rap = psC.tile([126, 512], F32, tag="rap")
                aap = psC.tile([126, 512], F32, tag="aap")
                nc.tensor.matmul(s1p[:, :], lhsT=ws1[:, 0:126], rhs=ft1[:, bs],
                                 start=True, stop=False)
                nc.tensor.matmul(s1p[:, :], lhsT=ws1[:, 126:252], rhs=ft2[:, bs],
                                 start=False, stop=True)
                nc.tensor.matmul(ccp[:, :], lhsT=wc[:, 0:126], rhs=ft1[:, bs],
                                 start=True, stop=False)
                nc.tensor.matmul(ccp[:, :], lhsT=wc[:, 126:252], rhs=ft2[:, bs],
                                 start=False, stop=True)
                nc.tensor.matmul(rap[:, :], lhsT=rep, rhs=ra66[:, bs], start=True, stop=True)
                nc.tensor.matmul(aap[:, :], lhsT=rep, rhs=a3sb[:, bs],
                                 start=True, stop=True)

                rsb = colpool.tile([126, 512], F32, tag="cD")
                nc.scalar.activation(rsb[:, :], rap[:, :], act.Copy)
                v = colpool.tile([126, 512], F32, tag="cA")
                nc.vector.tensor_tensor(v[:, :], s1p[:, :], rsb[:, :], op=alu.mult)
                # clamp(v,0,1)*10 via two Relus: r1=Relu(1-v); r2=Relu(10-10*r1)
                r1 = colpool.tile([126, 512], F32, tag="cB")
                nc.scalar.activation(r1[:, :], v[:, :], act.Relu, bias=1.0, scale=-1.0)
                t2 = colpool.tile([126, 512], F32, tag="cC")
                nc.scalar.activation(t2[:, :], r1[:, :], act.Relu, bias=10.0, scale=-10.0)
                # t2 := round(10*vc) + MAGIC
                nc.scalar.activation(t2[:, :], t2[:, :], act.Copy, bias=MAGIC, scale=1.0)
                kisb = colpool.tile([126, 512], F32, tag="cF")
                nc.scalar.activation(kisb[:, :], t2[:, :], act.Copy,
                                     bias=float(-MAGIC + 16 * iu), scale=1.0)
                # fv = 0.01*k^2*A - 0.2*k*S1 + C  (k = t2 - MAGIC)
                s2 = colpool.tile([126, 512], F32, tag="cB2")
                nc.vector.scalar_tensor_tensor(s2[:, :], t2[:, :], MAGIC, aap[:, :],
                                               op0=alu.subtract, op1=alu.mult)
                s3 = colpool.tile([126, 512], F32, tag="cC2")
                nc.vector.scalar_tensor_tensor(s3[:, :], s1p[:, :], -20.0, s2[:, :],
                                               op0=alu.mult, op1=alu.add)
                s4 = colpool.tile([126, 512], F32, tag="cD")
                nc.vector.scalar_tensor_tensor(s4[:, :], t2[:, :], MAGIC, s3[:, :],
                                               op0=alu.subtract, op1=alu.mult)
                fv = colpool.tile([126, 512], F32, tag="cE")
                nc.vector.scalar_tensor_tensor(fv[:, :], s4[:, :], 0.01, ccp[:, :],
                                               op0=alu.mult, op1=alu.add)
                cond = colpool.tile([126, 512], mybir.dt.uint8, tag="cG")
                nc.vector.tensor_tensor(cond[:, :], fv[:, :], mrun[:, bs], op=alu.is_lt)
                nc.vector.tensor_tensor(mrun[:, bs], mrun[:, bs], fv[:, :], op=alu.min)
                nc.vector.copy_predicated(out=ctrun[:, bs], mask=cond[:, :],
                                          data=kisb[:, :])

    ct = ctrun
    m32 = mrun

    # ---------------- decode iu*, k* ----------------
    iuf = tpool.tile([126, B_CORE], F32, tag="iuf")
    # iu+1 = rnd(ct/16 + 0.66875) ; (k-5.3)/16 in [-.331,+.294] avoids .5 ties
    nc.scalar.activation(iuf[:, :], ct[:, :], act.Copy, bias=0.66875, scale=0.0625)
    nc.scalar.activation(iuf[:, :], iuf[:, :], act.Copy, bias=MAGIC, scale=1.0)
    nc.scalar.activation(iuf[:, :], iuf[:, :], act.Copy, bias=-MAGIC - 1.0, scale=1.0)
    kst = tpool.tile([126, B_CORE], F32, tag="kst")
    nc.vector.scalar_tensor_tensor(kst[:, :], iuf[:, :], -16.0, ct[:, :],
                                   op0=alu.mult, op1=alu.add)
    uu = tpool.tile([126, B_CORE], F32, tag="uu")
    nc.scalar.activation(uu[:, :], iuf[:, :], act.Copy, bias=0.0, scale=0.1)
    vk = tpool.tile([126, B_CORE], F32, tag="vk")
    nc.scalar.activation(vk[:, :], kst[:, :], act.Copy, bias=0.0, scale=0.1)
    uv = tpool.tile([126, B_CORE], F32, tag="uv")
    nc.vector.tensor_tensor(uv[:, :], uu[:, :], vk[:, :], op=alu.mult)

    # ---------------- contact + normals ----------------
    nvec = tpool.tile([126, B_CORE * 3], BF16, tag="a3sb")
    n_v = nvec[:, :].rearrange("p (x b) -> p x b", x=3)
    vcx_all = tpool.tile([126, B_CORE * 3], BF16, tag="ft1")
    vcx_v = vcx_all[:, :].rearrange("p (x b) -> p x b", x=3)
    dvsb = tpool.tile([126, B_CORE * 3], BF16, tag="ft2")
    dv_v = dvsb[:, :].rearrange("p (x b) -> p x b", x=3)
    inner = tpool.tile([126, B_CORE], F32, tag="stats_sq")
    tmp = tpool.tile([126, B_CORE], F32, tag="tmp")

    with tc.tile_pool(name="psD", bufs=1, space="PSUM") as psD:
        for x in range(3):
            for h in range(2):
                bs = slice(h * 512, (h + 1) * 512)
                geo = []
                for g in range(5):
                    gk = (0, 1, 2, 3, 5)[g]
                    ps = psD.tile([126, 512], F32, tag=f"geo{g}")
                    nc.tensor.matmul(ps[:, :],
                                     lhsT=w_geo[:, (x * 6 + gk) * N_PAIR:(x * 6 + gk + 1) * N_PAIR],
                                     rhs=ft3[:, bs], start=True, stop=True)
                    geo.append(ps)
                t1x, t2x, t3x, t4x, dvx = geo
                nc.scalar.activation(dv_v[:, x, bs], dvx[:, :], act.Copy)
                q1 = colpool.tile([126, 512], F32, tag="cA")
                nc.vector.scalar_tensor_tensor(q1[:, :], uu[:, bs], 0.0, t2x[:, :],
                                               op0=alu.bypass, op1=alu.mult)
                q2 = colpool.tile([126, 512], F32, tag="cB")
                nc.vector.scalar_tensor_tensor(q2[:, :], vk[:, bs], 0.0, t3x[:, :],
                                               op0=alu.bypass, op1=alu.mult)
                q3 = colpool.tile([126, 512], F32, tag="cC")
                nc.vector.scalar_tensor_tensor(q3[:, :], uv[:, bs], 0.0, t4x[:, :],
                                               op0=alu.bypass, op1=alu.mult)
                y = colpool.tile([126, 512], F32, tag="cD")
                nc.vector.tensor_tensor(y[:, :], q1[:, :], q2[:, :], op=alu.add)
                nc.vector.tensor_tensor(y[:, :], y[:, :], q3[:, :], op=alu.add)
                nc.vector.tensor_tensor(vcx_v[:, x, bs], y[:, :],
                                        t1x[:, :], op=alu.add)

    # inner = sum_x vcx*dv  (accumulate)
    nc.vector.scalar_tensor_tensor(inner[:, :], vcx_v[:, 0, :], 0.0,
                                   dv_v[:, 0, :], op0=alu.bypass, op1=alu.mult)
    for x in (1, 2):
        nc.vector.scalar_tensor_tensor(tmp[:, :], vcx_v[:, x, :], 0.0,
                                       dv_v[:, x, :], op0=alu.bypass, op1=alu.mult)
        nc.vector.tensor_tensor(inner[:, :], inner[:, :], tmp[:, :], op=alu.add)
    w_t = tpool.tile([126, B_CORE], F32, tag="w_t")
    nc.vector.tensor_tensor(w_t[:, :], inner[:, :], rdvn2r[:, :], op=alu.mult)
    # n_x = vcx - w*dv ; nn accum
    nn = tpool.tile([126, B_CORE], F32, tag="iuf")
    for x in range(3):
        nc.vector.scalar_tensor_tensor(tmp[:, :], w_t[:, :], 0.0,
                                       dv_v[:, x, :], op0=alu.bypass, op1=alu.mult)
        nc.vector.tensor_tensor(n_v[:, x, :], vcx_v[:, x, :], tmp[:, :],
                                op=alu.subtract)
        nc.vector.scalar_tensor_tensor(tmp[:, :], n_v[:, x, :], 0.0, n_v[:, x, :],
                                       op0=alu.bypass, op1=alu.mult)
        if x == 0:
            nc.vector.tensor_copy(nn[:, :], tmp[:, :])
        else:
            nc.vector.tensor_tensor(nn[:, :], nn[:, :], tmp[:, :], op=alu.add)
    rn = tpool.tile([126, B_CORE], F32, tag="kst")
    nc.scalar.activation(rn[:, :], nn[:, :], act.Sqrt)
    nc.scalar.activation(rn[:, :], rn[:, :], act.Copy, bias=1e-5, scale=1.0)
    nc.vector.reciprocal_approx_fast(out=rn[:, :], in_=rn[:, :])

    # ---------------- selection (B-layout) + mask transpose back ----------------
    mask_t = tpool.tile([126, B_CORE], F32, tag="mask_t")
    with tc.tile_pool(name="psE", bufs=2, space="PSUM") as psE:
        for t in range(N_TILES):
            cs = slice(t * 128, (t + 1) * 128)
            mb = bpool.tile([128, 126], F32, tag="mb")
            pt = psE.tile([128, 128], F32, tag="tp")
            nc.tensor.transpose(out=pt[:, 0:126], in_=m32[:, cs], identity=ident[0:126, 0:126])
            nc.scalar.activation(mb[:, :], pt[:, 0:126], act.Copy)
            tb = bpool.tile([128, 1], F32, tag="tb")
            pt2 = psE.tile([128, 32], F32, tag="tp2")
            nc.tensor.transpose(out=pt2[:, 0:1], in_=tau2_t[:, cs], identity=ident[0:1, 0:1])
            nc.scalar.activation(tb[:, :], pt2[:, 0:1], act.Copy)

            neg = bpool.tile([128, 126], F32, tag="neg")
            nc.scalar.activation(neg[:, :], mb[:, :], act.Copy, bias=0.0, scale=-1.0)
            v8a = bpool.tile([128, 8], F32, tag="v8a")
            nc.vector.max(out=v8a[:, :], in_=neg[:, :])
            negr = bpool.tile([128, 126], F32, tag="negr")
            nc.vector.match_replace(out=negr[:, :], in_to_replace=v8a[:, :],
                                    in_values=neg[:, :], imm_value=-3.0e38)
            v8b = bpool.tile([128, 8], F32, tag="v8b")
            nc.vector.max(out=v8b[:, :], in_=negr[:, :])
            # mark the top-10 positions: replace top-8 (v8a) then ranks 9-10
            # (v8b cols 0:2; cols 2:8 neutralized) with +BIG; first-occurrence
            # semantics matches the reference's stable tie handling.
            nc.vector.memset(v8b[:, 2:8], -2.9e38)
            m1 = bpool.tile([128, 126], F32, tag="lt")
            nc.vector.match_replace(out=m1[:, :], in_to_replace=v8a[:, :],
                                    in_values=neg[:, :], imm_value=1.0e38)
            m2 = bpool.tile([128, 126], F32, tag="eq")
            nc.vector.match_replace(out=m2[:, :], in_to_replace=v8b[:, :],
                                    in_values=m1[:, :], imm_value=1.0e38)
            sel = bpool.tile([128, 126], F32, tag="cum")
            nc.vector.tensor_scalar(sel[:, :], m2[:, :], 9.0e37, None, op0=alu.is_ge)
            tcmp = bpool.tile([128, 126], F32, tag="tcmp")
            nc.vector.tensor_scalar(tcmp[:, :], mb[:, :], tb[:, 0:1], None, op0=alu.is_lt)
            mask = bpool.tile([128, 126], F32, tag="mask")
            nc.vector.tensor_tensor(mask[:, :], sel[:, :], tcmp[:, :], op=alu.mult)
            # transpose mask back to T: [126, 128]
            ptm = psE.tile([126, 128], F32, tag="tpm")
            nc.tensor.transpose(out=ptm[:, :], in_=mask[:, :], identity=ident[:, :])
            nc.scalar.activation(mask_t[:, cs], ptm[:, :], act.Copy)

    # ---------------- final contraction (T-layout) ----------------
    mrn = tpool.tile([126, B_CORE], F32, tag="uv")
    nc.vector.tensor_tensor(mrn[:, :], mask_t[:, :], rn[:, :], op=alu.mult)
    contrib = tpool.tile([126, B_CORE], F32, tag="uu")
    num_t = tpool.tile([1, B_CORE], F32, tag="num_t")
    den_t = tpool.tile([1, B_CORE], F32, tag="den_t")
    sx = []
    for x in range(3):
        sxt = tpool.tile([1, B_CORE], F32, tag=f"sx{x}")
        sx.append(sxt)
    with tc.tile_pool(name="psF", bufs=2, space="PSUM") as psF:
        for x in range(3):
            nc.vector.tensor_tensor(contrib[:, :], n_v[:, x, :],
                                    mrn[:, :], op=alu.mult)
            for h in range(2):
                bs = slice(h * 512, (h + 1) * 512)
                ps = psF.tile([1, 512], F32, tag="psx")
                nc.tensor.matmul(ps[:, :], lhsT=ones126[:, :], rhs=contrib[:, bs],
                                 start=True, stop=True)
                nc.scalar.activation(sx[x][:, bs], ps[:, :], act.Copy)
        for h in range(2):
            bs = slice(h * 512, (h + 1) * 512)
            ps = psF.tile([1, 512], F32, tag="psc")
            nc.tensor.matmul(ps[:, :], lhsT=ones126[:, :], rhs=mask_t[:, bs],
                             start=True, stop=True)
            nc.scalar.activation(den_t[:, bs], ps[:, :], act.Square)
    # num = Sx^2 + Sy^2 + Sz^2
    nc.vector.tensor_tensor(num_t[:, :], sx[0][:, :], sx[0][:, :], op=alu.mult)
    for x in (1, 2):
        nc.vector.scalar_tensor_tensor(sx[x][:, :], sx[x][:, :], 0.0, sx[x][:, :],
                                       op0=alu.bypass, op1=alu.mult)
        nc.vector.tensor_tensor(num_t[:, :], num_t[:, :], sx[x][:, :], op=alu.add)
    nc.sync.dma_start(out=d["out"][0:1, :], in_=num_t[:, :])
    nc.sync.dma_start(out=d["out"][1:2, :], in_=den_t[:, :])
    ctx.close()


# ---------------------------------------------------------------- host side

_CACHE = {}


def _get_compiled():
    if "nc" not in _CACHE:
        nc = bacc.Bacc("TRN2", target_bir_lowering=False, debug=False,
                       enable_asserts=False, num_devices=N_CORES)
        build_kernel(nc)
        nc.compile()
        _CACHE["nc"] = nc
    return _CACHE["nc"]


def kernel(poses: np.ndarray) -> np.ndarray:
    poses = np.asarray(poses, dtype=np.float32)
    bs = poses.shape[0]
    assert bs == B_CORE * N_CORES, f"expected {B_CORE * N_CORES}, got {bs}"
    consts = build_consts()
    nc = _get_compiled()
    in_maps = []
    for c in range(N_CORES):
        m = {"poses": poses[c * B_CORE:(c + 1) * B_CORE].reshape(B_CORE, 87).copy()}
        m.update(consts)
        in_maps.append(m)
    res = bass_utils.run_bass_kernel_spmd(nc, in_maps, core_ids=list(range(N_CORES)))
    num = 0.0
    den = 0.0
    for c in range(N_CORES):
        o = res.results[c]["out"]
        num += o[0, :].sum(dtype=np.float64)
        den += o[1, :].sum(dtype=np.float64)
    return np.float32(num / (den + 1.0))

